# revision 3
# baseline (speedup 1.0000x reference)
"""Trainium2 Bass kernel for nn_Attention_88321707475088.

GQA attention layer (S=2048, D=4096, 32 q-heads / 8 kv-heads, head_dim 128,
interleaved-pair RoPE, softmax, o-proj), tensor-parallel over heads across
8 NeuronCores. Each core owns 4 q-heads + 1 kv-head: wq/wk/wv sharded
column-wise, wo row-wise; partial outputs are summed on the host (the
all-reduce of the TP layout).

All matmuls run in fp32r (TensorE reduced-precision fp32 mode: full bf16-rate
for free dims >= 256, ~1.5e-4 matmul rel err).

Core dataflow (per core), designed so no on-device operand ever needs a
transpose except 16 tiny 128x128 PE transposes for V:
  - host supplies x^T blocked [32 ktiles, 4 schunks, 128, 512]
  - qT[h] [128d, 2048s] = wq_h^T @ x^T   (weights stationary, x^T moving)
  - kT    [128d, 2048s] = wk^T @ x^T ; vT = wv^T @ x^T -> PE-transpose -> v [t,d]
  - RoPE applied in [d, s] layout: host permutes wq/wk columns per head to
    [even dims, odd dims] so pairs are partition halves; cos/sin tables are
    duplicated into both halves -> all DVE ops have equal base partitions
  - scores^T[t,q] = kT_tile^T @ qT  (contraction over d = partitions)
  - E = exp(scores * 1/sqrt(128)) via ACT, fp32r
  - row sums via ones[128,128] matmul -> broadcast sums psum [128, 512]
  - out^T[d,q] += v_tile^T(E)  ; normalized by reciprocal_approx_fast
  - out_partial = out^T^T @ wo_shard  (lhsT = out^T, natural layout)
"""

import math

import numpy as np
import ml_dtypes

SEQ = 2048
DIM = 4096
N_HEADS = 32
HEAD_DIM = 128
N_KV_HEADS = 8
N_CORES = 8
ROPE_THETA = 10000.0

HL = N_HEADS // N_CORES          # 4 local q heads
MQ = HL * HEAD_DIM               # 512 local q columns
KT = DIM // 128                  # 32 contraction k-tiles
SC = 4                           # s-chunks in phase A (512 wide)
SCW = SEQ // SC                  # 512
TT = SEQ // 128                  # 16 t-tiles
QC = 4                           # q-chunks in phase B (512 wide)
QCW = SEQ // QC                  # 512
NCH = DIM // 512                 # 8 output dim chunks

_bf16 = ml_dtypes.bfloat16
_CACHE = {}


def _build():
    import concourse.mybir as mybir
    import concourse.tile as tile
    from concourse import bacc

    F32 = mybir.dt.float32
    F32R = mybir.dt.float32r
    AF = mybir.ActivationFunctionType

    nc = bacc.Bacc("TRN2", target_bir_lowering=False, debug=False,
                   num_devices=N_CORES)

    xt_d = nc.declare_dram_parameter("xt", [KT, SC, 128, SCW], F32R, isOutput=False)
    wq_d = nc.declare_dram_parameter("wq", [DIM, MQ], F32R, isOutput=False)
    wk_d = nc.declare_dram_parameter("wk", [DIM, HEAD_DIM], F32R, isOutput=False)
    wv_d = nc.declare_dram_parameter("wv", [DIM, HEAD_DIM], F32R, isOutput=False)
    wo_d = nc.declare_dram_parameter("wo", [NCH, HL, 128, 512], F32R, isOutput=False)
    cs_d = nc.declare_dram_parameter("cs", [128, SEQ], F32, isOutput=False)
    sn_d = nc.declare_dram_parameter("sn", [128, SEQ], F32, isOutput=False)
    ones_d = nc.declare_dram_parameter("ones", [128, 128], F32R, isOutput=False)
    ident_d = nc.declare_dram_parameter("ident", [128, 128], F32R, isOutput=False)
    out_d = nc.declare_dram_parameter("out", [SEQ, DIM], F32, isOutput=True)

    with tile.TileContext(nc) as tc:
        with tc.tile_pool(name="persist", bufs=1) as persist:
            ones_t = persist.tile([128, 128], F32R, name="ones")
            nc.sync.dma_start(ones_t, ones_d[:])
            ident_t = persist.tile([128, 128], F32R, name="ident")
            nc.sync.dma_start(ident_t, ident_d[:])
            outT = [persist.tile([128, SEQ], F32R, name=f"outT{h}")
                    for h in range(HL)]
            _run_phases(nc, tc, persist, ones_t, ident_t, outT, locals())
    nc.compile()
    return nc


def _run_phases(nc, tc, persist, ones_t, ident_t, outT, env):
    import concourse.mybir as mybir
    import concourse.tile as tile
    F32 = mybir.dt.float32
    F32R = mybir.dt.float32r
    AF = mybir.ActivationFunctionType
    xt_d, wq_d, wk_d, wv_d, wo_d = (env[k] for k in ["xt_d", "wq_d", "wk_d", "wv_d", "wo_d"])
    cs_d, sn_d, out_d = env["cs_d"], env["sn_d"], env["out_d"]

    if True:
        with tc.tile_pool(name="attn_in", bufs=1) as attn_in:
            # attention inputs (live through phase B)
            qT = [attn_in.tile([128, SEQ], F32R, name=f"qT{h}") for h in range(HL)]
            kT_sb = attn_in.tile([128, SEQ], F32R, name="kT")
            vS = attn_in.tile([128, TT, 128], F32R, name="vS")

            # ---------------- Phase A: projections + RoPE ----------------
            with tc.tile_pool(name="wqp", bufs=1) as wqp, \
                 tc.tile_pool(name="wkvp", bufs=1) as wkvp, \
                 tc.tile_pool(name="xa", bufs=3) as xa, \
                 tc.tile_pool(name="csp", bufs=1) as csp, \
                 tc.tile_pool(name="rtmp", bufs=1) as rtmp, \
                 tc.tile_pool(name="vtmp", bufs=2) as vtmp, \
                 tc.tile_pool(name="qps", bufs=1, space="PSUM") as qps, \
                 tc.tile_pool(name="kps", bufs=2, space="PSUM") as kps, \
                 tc.tile_pool(name="vps", bufs=1, space="PSUM") as vps, \
                 tc.tile_pool(name="vtr", bufs=1, space="PSUM") as vtr:
                wq_t = []
                for k in range(KT):
                    t = wqp.tile([128, MQ], F32R, name=f"wq{k}")
                    nc.sync.dma_start(t, wq_d[k * 128:(k + 1) * 128, :])
                    wq_t.append(t)
                wk_t = []
                wv_t = []
                for k in range(KT):
                    t = wkvp.tile([128, HEAD_DIM], F32R, name=f"wk{k}")
                    nc.sync.dma_start(t, wk_d[k * 128:(k + 1) * 128, :])
                    wk_t.append(t)
                    t = wkvp.tile([128, HEAD_DIM], F32R, name=f"wv{k}")
                    nc.sync.dma_start(t, wv_d[k * 128:(k + 1) * 128, :])
                    wv_t.append(t)
                for sc in range(SC):
                    ssl = slice(sc * SCW, (sc + 1) * SCW)
                    q_ps = [qps.tile([128, SCW], F32, name=f"q{m}") for m in range(HL)]
                    k_ps = kps.tile([128, SCW], F32, name="k")
                    v_ps = vps.tile([128, SCW], F32, name="v")
                    for k in range(KT):
                        x_t = xa.tile([128, SCW], F32R, name="x")
                        nc.sync.dma_start(x_t, xt_d[k, sc])
                        st = (k == 0)
                        sp = (k == KT - 1)
                        for m in range(HL):
                            nc.tensor.matmul(q_ps[m], lhsT=wq_t[k][:, m * 128:(m + 1) * 128],
                                             rhs=x_t, start=st, stop=sp)
                        nc.tensor.matmul(k_ps, lhsT=wk_t[k], rhs=x_t, start=st, stop=sp)
                        nc.tensor.matmul(v_ps, lhsT=wv_t[k], rhs=x_t, start=st, stop=sp)

                    # RoPE for q heads + k, emitted psum-reads-first per group
                    c_t = csp.tile([128, SCW], F32, name="c")
                    nc.sync.dma_start(c_t, cs_d[:, ssl])
                    s_t = csp.tile([128, SCW], F32, name="s")
                    nc.sync.dma_start(s_t, sn_d[:, ssl])

                    def rope(src_ps, dst):
                        x0 = src_ps[0:64, :]
                        x1 = src_ps[64:128, :]
                        t0 = rtmp.tile([64, SCW], F32, name="t0")
                        nc.vector.tensor_mul(t0, x0, c_t[0:64, :])
                        t1 = rtmp.tile([64, SCW], F32, name="t1")
                        nc.vector.tensor_mul(t1, x1, s_t[64:128, :])
                        t2 = rtmp.tile([64, SCW], F32, name="t2")
                        nc.vector.tensor_mul(t2, x0, s_t[0:64, :])
                        t3 = rtmp.tile([64, SCW], F32, name="t3")
                        nc.vector.tensor_mul(t3, x1, c_t[64:128, :])
                        nc.vector.tensor_sub(dst[0:64, :], t0, t1)
                        nc.vector.tensor_add(dst[64:128, :], t2, t3)

                    for m in range(HL):
                        rope(q_ps[m], qT[m][:, ssl])
                    rope(k_ps, kT_sb[:, ssl])

                    # v: copy vT psum -> sbuf, PE-transpose 128x128 blocks -> vS
                    v_sb = vtmp.tile([128, SCW], F32R, name="vsb")
                    nc.vector.tensor_copy(v_sb, v_ps)
                    vt_ps = vtr.tile([128, SCW // 128, 128], F32R, name="vt")
                    for j in range(SCW // 128):
                        nc.tensor.transpose(vt_ps[:, j, :], v_sb[:, j * 128:(j + 1) * 128],
                                            ident_t)
                    nc.vector.tensor_copy(vS[:, sc * (SCW // 128):(sc + 1) * (SCW // 128), :],
                                          vt_ps)

            # ---------------- Phase B: attention ----------------
            scale = 1.0 / math.sqrt(float(HEAD_DIM))
            with tc.tile_pool(name="ep", bufs=2) as ep, \
                 tc.tile_pool(name="rp", bufs=2) as rp, \
                 tc.tile_pool(name="scp", bufs=2, space="PSUM") as scp, \
                 tc.tile_pool(name="ops_", bufs=2, space="PSUM") as ops_, \
                 tc.tile_pool(name="sps", bufs=2, space="PSUM") as sps:
                units = [(h, qc) for qc in range(QC) for h in range(HL)]
                st_E = {}
                st_ops = {}
                st_sps = {}

                def emit_scores(i):
                    h, qc = units[i]
                    qsl = slice(qc * QCW, (qc + 1) * QCW)
                    E = ep.tile([128, TT, QCW], F32R, name="E")
                    st_E[i] = E
                    o_ps = ops_.tile([128, QCW], F32, name="o")
                    st_ops[i] = o_ps
                    s_ps = sps.tile([128, QCW], F32, name="s")
                    st_sps[i] = s_ps
                    for g in range(TT // 2):
                        sc_ps = scp.tile([128, 2, QCW], F32, name="sc")
                        for j in range(2):
                            t = 2 * g + j
                            nc.tensor.matmul(sc_ps[:, j, :],
                                             lhsT=kT_sb[:, t * 128:(t + 1) * 128],
                                             rhs=qT[h][:, qsl], start=True, stop=True)
                        nc.scalar.activation(E[:, 2 * g:2 * g + 2, :], sc_ps,
                                             AF.Exp, scale=scale)
                        yield g

                def emit_sums_av(i, g):
                    # two t-steps of the ones-sum and attn@v accumulations
                    E = st_E[i]
                    for j in range(2):
                        t = 2 * g + j
                        nc.tensor.matmul(st_sps[i], lhsT=ones_t, rhs=E[:, t, :],
                                         start=(t == 0), stop=(t == TT - 1))
                        nc.tensor.matmul(st_ops[i], lhsT=vS[:, t, :], rhs=E[:, t, :],
                                         start=(t == 0), stop=(t == TT - 1))

                def emit_norm(i):
                    h, qc = units[i]
                    qsl = slice(qc * QCW, (qc + 1) * QCW)
                    r_sb = rp.tile([128, QCW], F32, name="r")
                    nc.vector.reciprocal_approx_fast(r_sb, st_sps.pop(i))
                    nc.vector.tensor_mul(outT[h][:, qsl], st_ops.pop(i), r_sb)
                    st_E.pop(i)

                gens = {}
                for i in range(len(units) + 1):
                    if i >= 2:
                        emit_norm(i - 2)
                    if i < len(units):
                        gens[i] = emit_scores(i)
                        for g in gens[i]:
                            if i >= 1:
                                emit_sums_av(i - 1, g)
                    elif i - 1 >= 0:
                        for g in range(TT // 2):
                            emit_sums_av(i - 1, g)
                emit_norm(len(units) - 1)

            # ---------------- Phase C: output projection ----------------
            with tc.tile_pool(name="wop", bufs=2) as wop, \
                 tc.tile_pool(name="ost", bufs=3) as ost, \
                 tc.tile_pool(name="cps", bufs=4, space="PSUM") as cps:
                for nch in range(NCH):
                    wo_t = wop.tile([128, HL, 512], F32R, name="wo")
                    nc.sync.dma_start(wo_t, wo_d[nch].rearrange("h p n -> p h n"))
                    for stt in range(TT):
                        c_ps = cps.tile([128, 512], F32, name="c")
                        for h in range(HL):
                            nc.tensor.matmul(c_ps, lhsT=outT[h][:, stt * 128:(stt + 1) * 128],
                                             rhs=wo_t[:, h, :], start=(h == 0),
                                             stop=(h == HL - 1))
                        o_sb = ost.tile([128, 512], F32, name="osb")
                        nc.vector.tensor_copy(o_sb, c_ps)
                        nc.sync.dma_start(
                            out_d[stt * 128:(stt + 1) * 128, nch * 512:(nch + 1) * 512],
                            o_sb)


def _host_prep(x, wq, wk, wv, wo):
    """Build per-core input maps (all host-side numpy)."""
    f32 = np.float32
    x = np.asarray(x, dtype=f32)
    wq = np.asarray(wq, dtype=f32)
    wk = np.asarray(wk, dtype=f32)
    wv = np.asarray(wv, dtype=f32)
    wo = np.asarray(wo, dtype=f32)

    # x^T blocked [KT, SC, 128, SCW]
    xt = np.ascontiguousarray(
        x.T.reshape(KT, 128, SC, SCW).transpose(0, 2, 1, 3))

    # rope permutation within each head: [evens, odds]
    perm = np.concatenate([np.arange(0, HEAD_DIM, 2), np.arange(1, HEAD_DIM, 2)])

    inv = 1.0 / (ROPE_THETA ** (np.arange(0, HEAD_DIM, 2, dtype=f32) / HEAD_DIM))
    tpos = np.arange(SEQ, dtype=f32)
    ang = np.outer(tpos, inv)          # [S, 64]
    cosT = np.cos(ang).T               # [64, S]
    sinT = np.sin(ang).T
    cs = np.ascontiguousarray(np.concatenate([cosT, cosT], axis=0), dtype=f32)
    sn = np.ascontiguousarray(np.concatenate([sinT, sinT], axis=0), dtype=f32)

    ones = np.ones((128, 128), dtype=f32)
    ident = np.eye(128, dtype=f32)

    in_maps = []
    for c in range(N_CORES):
        wq_s = np.ascontiguousarray(
            wq[:, c * MQ:(c + 1) * MQ].reshape(DIM, HL, HEAD_DIM)[:, :, perm]
            .reshape(DIM, MQ))
        wk_s = np.ascontiguousarray(wk[:, c * HEAD_DIM:(c + 1) * HEAD_DIM][:, perm])
        wv_s = np.ascontiguousarray(wv[:, c * HEAD_DIM:(c + 1) * HEAD_DIM])
        wo_s = wo[c * MQ:(c + 1) * MQ, :]          # [512, 4096]
        wo_b = np.ascontiguousarray(
            wo_s.reshape(HL, 128, NCH, 512).transpose(2, 0, 1, 3))  # [NCH, HL, 128, 512]
        in_maps.append({
            "xt": xt, "wq": wq_s, "wk": wk_s, "wv": wv_s, "wo": wo_b,
            "cs": cs, "sn": sn, "ones": ones, "ident": ident,
        })
    return in_maps


def kernel(x, wq, wk, wv, wo):
    from concourse.bass_utils import run_bass_kernel_spmd

    if "nc" not in _CACHE:
        _CACHE["nc"] = _build()
    nc = _CACHE["nc"]

    in_maps = _host_prep(x, wq, wk, wv, wo)
    res = run_bass_kernel_spmd(nc, in_maps, list(range(N_CORES)))
    out = res.results[0]["out"].astype(np.float32, copy=True)
    for c in range(1, N_CORES):
        out += res.results[c]["out"]
    return out
